# revision 3
# baseline (speedup 1.0000x reference)
# kernel.py — fused causal ReLU-attention (qkv proj + q@k^T + relu/causal + @v)
# for Trainium2, 8 NeuronCores, batch-parallel (1 batch element per core).
#
# v2 design (120.0us -> target ~113us):
#  - host pre-transposes x (device gets x^T) and the device returns y^T;
#  - input DMAs issued k-ordered, each k-set split across BOTH HWDGE
#    queues (sync/scalar) in ~128-192KB pieces so k-set latency halves;
#    Wv columns ride the gpsimd SWDGE queue in parallel;
#  - the qk projection runs as 8-chain waves; the first 16 chains split
#    their k-accumulation (k0-2 -> f32 partial in SBUF via ACT, k3-5 +
#    combine via DVE) so the PE starts real work on the first k-set and
#    never holds all 8 PSUM banks across the whole load;
#  - windows carry no qk fills (projection done in waves); instead the
#    prev chunk's AV interleaves 1:1 with this chunk's score strips and
#    v-projection chains fill the early windows;
#  - chunk order: all c=0 chunks, then c=1, ending on the small (5,0)
#    chunk; final writeback split into 4 column pieces + 4 DMAs.
#
# Self-contained: hardcodes shapes B,T,C = 8,1024,768, nh=12, hs=64.
import os
import sys

for p in ("/opt/trn_rl_repo", "/root/.axon_site", "/root/.axon_site/_ro/trn_rl_repo"):
    if os.path.isdir(p) and p not in sys.path:
        sys.path.append(p)

import numpy as np

import concourse.bass as bass
import concourse.mybir as mybir
import concourse.tile as tile
from concourse import bacc
from concourse import bass_utils

F32 = mybir.dt.float32
BF16 = mybir.dt.bfloat16
AF = mybir.ActivationFunctionType
ALU = mybir.AluOpType

B, T, C = 8, 1024, 768
NH, HS = 12, 64
SCALE = 1.0 / 8.0  # 1/sqrt(64)
P = 128
NT = T // P      # 8 t-tiles
KC = C // P      # 6 c-tiles (contraction)
NPAIR = NH // 2  # 6 head pairs (one 128-wide m-tile per pair)
TCH = 512        # q-chunk width (one PSUM bank)
NCH = T // TCH   # 2 chunks
WQK = 2 * C      # first 1536 w cols are q|k


def build_nc(n_cores=8):
    nc = bacc.Bacc("TRN2", target_bir_lowering=False, debug=False,
                   num_devices=n_cores)

    xt_d = nc.dram_tensor("x", [C, T], BF16, kind="ExternalInput").ap()
    w_d = nc.dram_tensor("w", [C, 3 * C], BF16, kind="ExternalInput").ap()
    b_d = nc.dram_tensor("b", [3 * C], F32, kind="ExternalInput").ap()
    y_d = nc.dram_tensor("y", [C, T], BF16, kind="ExternalOutput").ap()

    with tile.TileContext(nc) as tc:
        _emit(nc, tc, xt_d, w_d, b_d, y_d)

    nc.compile()
    return nc


def _emit(nc, tc, xt_d, w_d, b_d, y_d):
    from contextlib import ExitStack

    with ExitStack() as ctx:
        pp = ctx.enter_context(tc.tile_pool(name="persist", bufs=1))
        xtp = ctx.enter_context(tc.tile_pool(name="xT", bufs=1))
        wp = ctx.enter_context(tc.tile_pool(name="w", bufs=1))
        ptp = ctx.enter_context(tc.tile_pool(name="part", bufs=1))
        vp = ctx.enter_context(tc.tile_pool(name="vsb", bufs=1))
        at_pool = ctx.enter_context(tc.tile_pool(name="attp", bufs=32))
        yt_pool = ctx.enter_context(tc.tile_pool(name="yT", bufs=3))
        qps = ctx.enter_context(
            tc.tile_pool(name="qkv_psum", bufs=2, space="PSUM"))
        a_ps = ctx.enter_context(
            tc.tile_pool(name="att_psum", bufs=4, space="PSUM"))
        y_ps = ctx.enter_context(
            tc.tile_pool(name="y_psum", bufs=2, space="PSUM"))

        # ---- persistent tiles ----
        xT = [xtp.tile([P, T], BF16, tag=f"xT{k}", name=f"xT{k}")
              for k in range(KC)]
        w_sb = [wp.tile([P, 3 * C], BF16, tag=f"w{k}", name=f"w{k}")
                for k in range(KC)]
        bqk = pp.tile([P, 12], F32, tag="bqk", name="bqk")
        bv_row = pp.tile([1, C], F32, tag="bvrow", name="bvrow")
        bv = pp.tile([P, C], F32, tag="bv", name="bv")
        qkT = [pp.tile([P, T], BF16, tag=f"qkT{m}", name=f"qkT{m}")
               for m in range(2 * NPAIR)]
        v_sb = [vp.tile([P, C], BF16, tag=f"v{i}", name=f"v{i}")
                for i in range(NT)]
        parts = [ptp.tile([P, TCH], F32, tag=f"pt{j}", name=f"pt{j}")
                 for j in range(16)]

        def wslice(k, m):
            return w_sb[k][:, P * m:P * (m + 1)]

        # scratch memset FIRST on DVE so the HAM warm-up chain can start
        # the moment the tensor engine clears the framework preamble
        scratch = pp.tile([P, TCH], BF16, tag="scratch", name="scratch")
        nc.vector.memset(scratch[:], 0.0)

        # ---- input DMAs: k-ordered pieces alternating the two HWDGE
        # queues so each k-set (x_k halves + w_k qk-halves) lands with
        # half the single-queue latency; x t=1 halves trail by ~2 sets.
        def dma_piece(eng, kind, k):
            if kind == "xa":
                eng.dma_start(xT[k][:, 0:TCH], xt_d[P * k:P * (k + 1), 0:TCH])
            elif kind == "xb":
                eng.dma_start(xT[k][:, TCH:T], xt_d[P * k:P * (k + 1), TCH:T])
            elif kind == "wa":
                eng.dma_start(w_sb[k][:, 0:C], w_d[P * k:P * (k + 1), 0:C])
            elif kind == "wb":
                eng.dma_start(w_sb[k][:, C:WQK], w_d[P * k:P * (k + 1), C:WQK])

        order = []
        for k in range(KC):
            order += [("xa", k), ("wa", k), ("wb", k)]
            if k >= 1:
                order.append(("xb", k - 1))
        order.append(("xb", KC - 1))
        for idx, (kind, k) in enumerate(order):
            dma_piece(nc.sync if idx % 2 == 0 else nc.scalar, kind, k)

        # gpsimd SWDGE: biases + Wv columns (needed only by the v fills
        # deep in the window phase — plenty of slack on the slow queue)
        nc.gpsimd.dma_start(bqk[:],
                            b_d[0:WQK].rearrange("(a p) -> p a", p=P))
        nc.gpsimd.dma_start(bv_row[:],
                            b_d[WQK:3 * C].rearrange("(o a) -> o a", o=1))
        for k in range(KC):
            nc.gpsimd.dma_start(w_sb[k][:, WQK:3 * C],
                                w_d[P * k:P * (k + 1), WQK:3 * C])

        # ---- constants (gpsimd; after its DMA issues) ----
        # causal+scale mask for diagonal-start pieces:
        # cols [0,128) = (col >= part ? SCALE : 0) ; cols [128,640) = SCALE
        mask = pp.tile([P, 128 + TCH], F32, tag="mask", name="mask")
        nc.gpsimd.memset(mask[:], SCALE)
        nc.gpsimd.affine_select(
            out=mask[:, 0:P], in_=mask[:, 0:P],
            compare_op=ALU.is_ge, fill=0.0, base=0,
            pattern=[[1, P]], channel_multiplier=-1)
        # 0/1 wedge (bf16) for the two-op diag variant (ACT relu + DVE fix)
        wedge = pp.tile([P, P], BF16, tag="wedge", name="wedge")
        nc.gpsimd.memset(wedge[:], 1.0)
        nc.gpsimd.affine_select(
            out=wedge[:], in_=wedge[:],
            compare_op=ALU.is_ge, fill=0.0, base=0,
            pattern=[[1, P]], channel_multiplier=-1)
        nc.gpsimd.partition_broadcast(bv[:], bv_row[0:1, :])

        # ---- HAM warm-up dummy chain (bridges preamble -> first k-set)
        dps = qps.tile([P, TCH], F32, tag="qkvps", name="dummy")
        for r in range(8):
            nc.tensor.matmul(dps[:], scratch[:, 0:P], scratch[:],
                             start=(r == 0), stop=(r == 7))

        # ---- qk projection waves ----
        # chain = (m, t): qkT[m][:, 512t:512(t+1)] = W_m^T @ x^T + b_m
        CH_A = [(0, 0), (6, 0), (1, 0), (7, 0), (2, 0), (8, 0), (3, 0), (9, 0)]
        CH_A2 = [(4, 0), (10, 0), (5, 0), (11, 0), (0, 1), (6, 1), (1, 1), (7, 1)]
        CH_C = [(2, 1), (8, 1), (3, 1), (9, 1), (4, 1), (10, 1), (5, 1), (11, 1)]

        def bank(j):
            if j < 2:
                return qps.tile([P, TCH], F32, tag="qkvps", name="wv_ps")
            if j < 6:
                return a_ps.tile([P, TCH], F32, tag="aps", name="wv_ps")
            return y_ps.tile([P, TCH], F32, tag="yps", name="wv_ps")

        def wave_partA(chains, part_off):
            # k=0..2 k-major, then ACT evacuates psum+bias -> f32 partial
            ps = {}
            for k in (0, 1, 2):
                for j, (m, t) in enumerate(chains):
                    if k == 0:
                        ps[j] = bank(j)
                    nc.tensor.matmul(
                        ps[j][:], wslice(k, m),
                        xT[k][:, TCH * t:TCH * (t + 1)],
                        start=(k == 0), stop=(k == 2))
            for j, (m, t) in enumerate(chains):
                nc.scalar.activation(parts[part_off + j][:], ps[j][:],
                                     AF.Identity, bias=bqk[:, m:m + 1])

        def wave_partB(chains, part_off):
            # k=3..5 k-major, then DVE combines with the partial -> bf16
            ps = {}
            for k in (3, 4, 5):
                for j, (m, t) in enumerate(chains):
                    if k == 3:
                        ps[j] = bank(j)
                    nc.tensor.matmul(
                        ps[j][:], wslice(k, m),
                        xT[k][:, TCH * t:TCH * (t + 1)],
                        start=(k == 3), stop=(k == 5))
            for j, (m, t) in enumerate(chains):
                nc.vector.tensor_tensor(
                    qkT[m][:, TCH * t:TCH * (t + 1)], ps[j][:],
                    parts[part_off + j][:], ALU.add)

        def wave_whole(chains):
            ps = {}
            for k in range(KC):
                for j, (m, t) in enumerate(chains):
                    if k == 0:
                        ps[j] = bank(j)
                    nc.tensor.matmul(
                        ps[j][:], wslice(k, m),
                        xT[k][:, TCH * t:TCH * (t + 1)],
                        start=(k == 0), stop=(k == KC - 1))
            for j, (m, t) in enumerate(chains):
                nc.scalar.activation(qkT[m][:, TCH * t:TCH * (t + 1)],
                                     ps[j][:], AF.Identity,
                                     bias=bqk[:, m:m + 1])

        wave_partA(CH_A, 0)
        wave_partA(CH_A2, 8)
        wave_partB(CH_A, 0)
        wave_partB(CH_A2, 8)

        # ---- v projection: chains of 6 shared-stationary k-steps ----
        load = {"act": 0.0, "dve": 0.0}

        def v_steps(i):
            # v_sb[i] = x[128i:128(i+1), :] @ Wv + bv, as per-k closures
            box = {}
            pool, tag = (qps, "qkvps") if i % 2 == 0 else (a_ps, "aps")

            def step(k):
                def go():
                    if "pa" not in box:
                        box["pa"] = pool.tile([P, TCH], F32, tag=tag,
                                              name="vps_a")
                        box["pb"] = pool.tile([P, TCH], F32, tag=tag,
                                              name="vps_b")
                    xs = xT[k][:, P * i:P * (i + 1)]
                    nc.tensor.matmul(box["pa"][:], xs,
                                     w_sb[k][:, WQK:WQK + TCH],
                                     start=(k == 0), stop=(k == KC - 1))
                    nc.tensor.matmul(box["pb"][:, 0:C - TCH], xs,
                                     w_sb[k][:, WQK + TCH:3 * C],
                                     start=(k == 0), stop=(k == KC - 1))
                    if k == KC - 1:
                        nc.vector.tensor_tensor(
                            v_sb[i][:, 0:TCH], box["pa"][:],
                            bv[:, 0:TCH], ALU.add)
                        nc.vector.tensor_tensor(
                            v_sb[i][:, TCH:C], box["pb"][:, 0:C - TCH],
                            bv[:, TCH:C], ALU.add)
                        load["dve"] += 1063
                return go
            return [step(k) for k in range(KC)]

        # v0-3 must be done before the first AV (window 2): run them as a
        # block between the waves (Wv cols have landed by then)
        for i in range(4):
            for go in v_steps(i):
                go()

        wave_whole(CH_C)

        # ======= attention windows =======
        def relu_piece(at, ps, n, diag):
            # pick cheapest placement for the PSUM->SBUF relu pass
            act_c = n * 0.833 + 260
            dve_c = n * 1.042 + 130
            if not diag:
                if load["act"] + act_c <= load["dve"] + dve_c:
                    load["act"] += act_c
                    nc.scalar.activation(at[:, 0:n], ps[:, 0:n],
                                         AF.Relu, scale=SCALE)
                else:
                    load["dve"] += dve_c
                    nc.vector.tensor_scalar(
                        at[:, 0:n], ps[:, 0:n], SCALE, 0.0,
                        ALU.mult, ALU.max)
                return
            # diag: one-pass masked STT on DVE, or ACT relu + DVE wedge fix
            if load["dve"] + dve_c <= load["act"] + act_c + 200:
                load["dve"] += dve_c
                nc.vector.scalar_tensor_tensor(
                    at[:, 0:n], ps[:, 0:n], 0.0,
                    mask[:, 0:n], ALU.max, ALU.mult)
            else:
                load["act"] += act_c
                load["dve"] += 200
                nc.scalar.activation(at[:, 0:n], ps[:, 0:n],
                                     AF.Relu, scale=SCALE)
                nc.vector.tensor_tensor(at[:, 0:P], at[:, 0:P],
                                        wedge[:], ALU.mult)

        def make_chunk(p, c):
            qt, kt = qkT[p], qkT[NPAIR + p]
            c_lo, c_hi = TCH * c, TCH * (c + 1)
            nstr = 4 * c + 4
            pieces = {}

            def emit_score(r):
                q0 = max(P * r, c_lo)
                n = c_hi - q0
                diag = P * r >= c_lo
                prs = []
                for hh in range(2):
                    h0 = HS * hh
                    ps = a_ps.tile([P, TCH], F32, tag="aps", name="aps")
                    nc.tensor.matmul(
                        ps[:, 0:n],
                        kt[h0:h0 + HS, P * r:P * (r + 1)],
                        qt[h0:h0 + HS, q0:c_hi],
                        start=True, stop=True,
                        tile_position=(h0, 0))
                    at = at_pool.tile([P, TCH], BF16, tag="attp",
                                      name="attp")
                    relu_piece(at, ps, n, diag)
                    prs.append((q0 - c_lo, n, at))
                pieces[r] = prs

            state = {"yp": None}

            def emit_av(r):
                if state["yp"] is None:
                    state["yp"] = [y_ps.tile([P, TCH], F32, tag="yps",
                                             name="yps") for _ in range(2)]
                yp = state["yp"]
                for hh in range(2):
                    h0 = HS * hh
                    off, n, at = pieces[r][hh]
                    nc.tensor.matmul(
                        yp[hh][h0:h0 + HS, off:off + n],
                        v_sb[r][:, P * p + h0:P * p + h0 + HS],
                        at[:, 0:n],
                        start=(r == 0), stop=(r == nstr - 1),
                        tile_position=(0, h0))

            def emit_wb(c_, final=False):
                # y^T chunk PSUM -> SBUF (cast bf16) -> HBM
                yp = state["yp"]
                c_lo_, c_hi_ = TCH * c_, TCH * (c_ + 1)
                yt = yt_pool.tile([P, TCH], BF16, tag="yT", name="yT")
                if not final:
                    nc.scalar.activation(yt[0:HS, :], yp[0][0:HS, :],
                                         AF.Copy)
                    nc.vector.tensor_copy(yt[HS:P, :], yp[1][HS:P, :])
                    load["act"] += 687
                    load["dve"] += 660
                    (nc.sync if c_ == 0 else nc.scalar).dma_start(
                        y_d[P * p:P * (p + 1), c_lo_:c_hi_], yt[:, 0:TCH])
                else:
                    # split 4 ways so the last bytes hit HBM sooner
                    for q in range(4):
                        s0, s1 = P * q, P * (q + 1)
                        nc.scalar.activation(yt[0:HS, s0:s1],
                                             yp[0][0:HS, s0:s1], AF.Copy)
                        nc.vector.tensor_copy(yt[HS:P, s0:s1],
                                              yp[1][HS:P, s0:s1])
                        (nc.sync if q % 2 == 0 else nc.scalar).dma_start(
                            y_d[P * p:P * (p + 1), c_lo_ + s0:c_lo_ + s1],
                            yt[:, s0:s1])

            return list(range(nstr)), emit_score, emit_av, emit_wb

        # chunk stream: all c=0 first (AV then only needs v0-3), ending
        # on the small 4-strip (5,0) chunk so the drain is minimal
        chunks = [(p, 0) for p in range(NPAIR - 1)]
        chunks += [(p, 1) for p in range(NPAIR)]
        chunks.append((NPAIR - 1, 0))

        # v4-7 chains become fill work for the early windows
        fills = []
        for i in range(4, NT):
            fills.extend(v_steps(i))
        fi = 0

        prev = None  # (p, c, strips, emit_av, emit_wb)
        for (p, c) in chunks:
            strips, emit_score, emit_av, emit_wb = make_chunk(p, c)
            pstrips = prev[2] if prev else []
            nslot = len(strips)
            for si, r in enumerate(strips):
                emit_score(r)
                if si < len(pstrips):
                    prev[3](pstrips[si])
                for _ in range(2):
                    if fi < len(fills):
                        fills[fi]()
                        fi += 1
            # leftover prev-AV strips (prev chunk bigger than this one)
            for r in pstrips[nslot:]:
                prev[3](r)
            if prev:
                prev[4](prev[1])
            prev = (p, c, strips, emit_av, emit_wb)
        while fi < len(fills):
            fills[fi]()
            fi += 1
        # drain last chunk
        for r in prev[2]:
            prev[3](r)
        prev[4](prev[1], final=True)


def _ensure_ntff_hook():
    """Register the axon NTFF profiling hook if the image's antenv lacks
    axon_hooks (bass_utils hard-imports it on the trace=True path)."""
    import types
    try:
        from antenv import axon_hooks  # noqa: F401
        return
    except ImportError:
        pass
    import antenv
    mod = types.ModuleType("antenv.axon_hooks")
    mod._hook = None

    def set_axon_ntff_profile_hook(h):
        mod._hook = h

    def get_axon_ntff_profile_hook():
        return mod._hook

    mod.set_axon_ntff_profile_hook = set_axon_ntff_profile_hook
    mod.get_axon_ntff_profile_hook = get_axon_ntff_profile_hook
    sys.modules["antenv.axon_hooks"] = mod
    antenv.axon_hooks = mod
    try:
        from trn_agent_boot.trn_boot import _ntff_profile_via_ctypes
        hook = _ntff_profile_via_ctypes("/opt/axon/libaxon_pjrt.so")
        if hook is not None:
            mod._hook = hook
    except Exception:
        pass


_NC_CACHE = None


def _get_nc():
    global _NC_CACHE
    if _NC_CACHE is None:
        _NC_CACHE = build_nc()
    return _NC_CACHE


def kernel(x, W_attn, b_attn, _trace=False):
    import ml_dtypes
    x = np.asarray(x).astype(ml_dtypes.bfloat16)
    xt = np.ascontiguousarray(x.transpose(0, 2, 1))  # [B, C, T]
    w = np.ascontiguousarray(np.asarray(W_attn).astype(ml_dtypes.bfloat16))
    b = np.ascontiguousarray(np.asarray(b_attn, dtype=np.float32))
    assert xt.shape == (B, C, T) and w.shape == (C, 3 * C) and b.shape == (3 * C,)

    if _trace:
        _ensure_ntff_hook()
    nc = _get_nc()
    in_maps = [{"x": xt[i], "w": w, "b": b} for i in range(B)]
    res = bass_utils.run_bass_kernel_spmd(
        nc, in_maps, core_ids=list(range(B)), trace=_trace)
    y = np.stack([np.asarray(res.results[i]["y"]).astype(np.float32).T
                  for i in range(B)], axis=0)
    if _trace:
        kernel.last_result = res
    return y


# revision 4
# speedup vs baseline: 1.0899x; 1.0899x over previous
# kernel.py — fused causal ReLU-attention (qkv proj + q@k^T + relu/causal + @v)
# for Trainium2, 8 NeuronCores, batch-parallel (1 batch element per core).
#
# v3 design (from the 120.0us v1):
#  - host pre-transposes x (device gets x^T) and the device returns y^T;
#  - input DMAs issued as k-ordered ~128-192KB pieces alternating the two
#    HWDGE queues, so each k-set (x_k halves + w_k qk-halves) lands with
#    half the single-queue latency; all Wv tiles ride the gpsimd SWDGE
#    queue from the start (v runs ~20us in, plenty of slack);
#  - qk projection in two 7/8-chain k-major waves keeps the PE on real
#    work through the whole input load (no dead stall, HAM stays warm);
#  - v0-3 projected right after the waves; v4-7 and the remaining five
#    qk t1 chains become fill work interleaved into the score windows;
#  - chunk order: all c=0 chunks first, then c=1, ending on the small
#    4-strip (5,0) chunk; final writeback split into 4 independent
#    tiles/DMAs so the last bytes hit HBM with minimal serialization;
#  - PSUM: dum(1)+qkv(1)+att(4)+y(2) banks — fills use dum/qkv only and
#    never contend with the score/AV rotation.
#
# Self-contained: hardcodes shapes B,T,C = 8,1024,768, nh=12, hs=64.
import os
import sys

for p in ("/opt/trn_rl_repo", "/root/.axon_site", "/root/.axon_site/_ro/trn_rl_repo"):
    if os.path.isdir(p) and p not in sys.path:
        sys.path.append(p)

import numpy as np

import concourse.bass as bass
import concourse.mybir as mybir
import concourse.tile as tile
from concourse import bacc
from concourse import bass_utils

F32 = mybir.dt.float32
BF16 = mybir.dt.bfloat16
AF = mybir.ActivationFunctionType
ALU = mybir.AluOpType

B, T, C = 8, 1024, 768
NH, HS = 12, 64
SCALE = 1.0 / 8.0  # 1/sqrt(64)
P = 128
NT = T // P      # 8 t-tiles
KC = C // P      # 6 c-tiles (contraction)
NPAIR = NH // 2  # 6 head pairs
TCH = 512        # q-chunk width (one PSUM bank)
NCH = T // TCH   # 2 chunks
WQK = 2 * C      # first 1536 w cols are q|k


def build_nc(n_cores=8):
    nc = bacc.Bacc("TRN2", target_bir_lowering=False, debug=False,
                   num_devices=n_cores)

    xt_d = nc.dram_tensor("x", [C, T], BF16, kind="ExternalInput").ap()
    w_d = nc.dram_tensor("w", [C, 3 * C], BF16, kind="ExternalInput").ap()
    b_d = nc.dram_tensor("b", [3 * C], F32, kind="ExternalInput").ap()
    y_d = nc.dram_tensor("y", [C, T], BF16, kind="ExternalOutput").ap()

    with tile.TileContext(nc) as tc:
        _emit(nc, tc, xt_d, w_d, b_d, y_d)

    nc.compile()
    return nc


def _emit(nc, tc, xt_d, w_d, b_d, y_d):
    from contextlib import ExitStack

    with ExitStack() as ctx:
        pp = ctx.enter_context(tc.tile_pool(name="persist", bufs=1))
        xtp = ctx.enter_context(tc.tile_pool(name="xT", bufs=1))
        wqk_p = ctx.enter_context(tc.tile_pool(name="wqk", bufs=1))
        wv_p = ctx.enter_context(tc.tile_pool(name="wv", bufs=1))
        vp = ctx.enter_context(tc.tile_pool(name="vsb", bufs=1))
        at_pool = ctx.enter_context(tc.tile_pool(name="attp", bufs=24))
        yt_pool = ctx.enter_context(tc.tile_pool(name="yT", bufs=4))
        dum_ps = ctx.enter_context(
            tc.tile_pool(name="dum_psum", bufs=1, space="PSUM"))
        qps = ctx.enter_context(
            tc.tile_pool(name="qkv_psum", bufs=1, space="PSUM"))
        a_ps = ctx.enter_context(
            tc.tile_pool(name="att_psum", bufs=4, space="PSUM"))
        y_ps = ctx.enter_context(
            tc.tile_pool(name="y_psum", bufs=2, space="PSUM"))

        # ---- persistent tiles ----
        xT = [xtp.tile([P, T], BF16, tag=f"xT{k}", name=f"xT{k}")
              for k in range(KC)]
        w_sb = [wqk_p.tile([P, WQK], BF16, tag=f"w{k}", name=f"w{k}")
                for k in range(KC)]
        wv = [wv_p.tile([P, C], BF16, tag=f"wv{k}", name=f"wv{k}")
              for k in range(KC)]
        bqk = pp.tile([P, 12], F32, tag="bqk", name="bqk")
        bv_row = pp.tile([1, C], F32, tag="bvrow", name="bvrow")
        bv = pp.tile([P, C], F32, tag="bv", name="bv")
        qkT = [pp.tile([P, T], BF16, tag=f"qkT{m}", name=f"qkT{m}")
               for m in range(2 * NPAIR)]
        v_sb = [vp.tile([P, C], BF16, tag=f"v{i}", name=f"v{i}")
                for i in range(NT)]

        def wslice(k, m):
            return w_sb[k][:, P * m:P * (m + 1)]

        # scratch memset FIRST on DVE so the HAM warm-up chain can start
        # the moment the tensor engine clears the framework preamble
        scratch = pp.tile([P, TCH], BF16, tag="scratch", name="scratch")
        nc.vector.memset(scratch[:], 0.0)

        # ---- input DMAs: k-ordered pieces alternating the HWDGE queues
        def dma_piece(eng, kind, k):
            if kind == "xa":
                eng.dma_start(xT[k][:, 0:TCH], xt_d[P * k:P * (k + 1), 0:TCH])
            elif kind == "xb":
                eng.dma_start(xT[k][:, TCH:T], xt_d[P * k:P * (k + 1), TCH:T])
            elif kind == "wa":
                eng.dma_start(w_sb[k][:, 0:C], w_d[P * k:P * (k + 1), 0:C])
            elif kind == "wb":
                eng.dma_start(w_sb[k][:, C:WQK], w_d[P * k:P * (k + 1), C:WQK])

        order = []
        for k in range(KC):
            order += [("xa", k), ("wa", k), ("wb", k)]
            if k >= 1:
                order.append(("xb", k - 1))
        order.append(("xb", KC - 1))
        for idx, (kind, k) in enumerate(order):
            dma_piece(nc.sync if idx % 2 == 0 else nc.scalar, kind, k)

        # gpsimd SWDGE: biases + all Wv tiles (v runs ~20us in)
        nc.gpsimd.dma_start(bqk[:],
                            b_d[0:WQK].rearrange("(a p) -> p a", p=P))
        nc.gpsimd.dma_start(bv_row[:],
                            b_d[WQK:3 * C].rearrange("(o a) -> o a", o=1))
        for k in range(KC):
            nc.gpsimd.dma_start(wv[k][:], w_d[P * k:P * (k + 1), WQK:3 * C])

        # ---- constants (gpsimd; after its DMA issues) ----
        mask = pp.tile([P, 128 + TCH], F32, tag="mask", name="mask")
        nc.gpsimd.memset(mask[:], SCALE)
        nc.gpsimd.affine_select(
            out=mask[:, 0:P], in_=mask[:, 0:P],
            compare_op=ALU.is_ge, fill=0.0, base=0,
            pattern=[[1, P]], channel_multiplier=-1)
        wedge = pp.tile([P, P], BF16, tag="wedge", name="wedge")
        nc.gpsimd.memset(wedge[:], 1.0)
        nc.gpsimd.affine_select(
            out=wedge[:], in_=wedge[:],
            compare_op=ALU.is_ge, fill=0.0, base=0,
            pattern=[[1, P]], channel_multiplier=-1)
        nc.gpsimd.partition_broadcast(bv[:], bv_row[0:1, :])

        # ---- HAM warm-up dummy chain on the dedicated dum bank ----
        dps = dum_ps.tile([P, TCH], F32, tag="dum", name="dum")
        for r in range(8):
            nc.tensor.matmul(dps[:], scratch[:, 0:P], scratch[:],
                             start=(r == 0), stop=(r == 7))

        # ---- qk projection waves (k-major; chain = (m, t)) ----
        # wave-1: 7 chains so the dum bank stays free during the load
        W1 = [(0, 0), (6, 0), (1, 0), (7, 0), (2, 0), (8, 0), (3, 0)]
        W2 = [(9, 0), (4, 0), (10, 0), (5, 0), (7, 1), (0, 1), (6, 1), (1, 1)]

        def bank(j):
            # j in 0..7 -> qps(1) + a_ps(4) + y_ps(2) + dum(1)
            if j == 0:
                return qps.tile([P, TCH], F32, tag="qkvps", name="wps")
            if j < 5:
                return a_ps.tile([P, TCH], F32, tag="aps", name="wps")
            if j < 7:
                return y_ps.tile([P, TCH], F32, tag="yps", name="wps")
            return dum_ps.tile([P, TCH], F32, tag="dum", name="wps")

        def wave(chains):
            ps = {}
            for k in range(KC):
                for j, (m, t) in enumerate(chains):
                    if k == 0:
                        ps[j] = bank(j)
                    nc.tensor.matmul(
                        ps[j][:], wslice(k, m),
                        xT[k][:, TCH * t:TCH * (t + 1)],
                        start=(k == 0), stop=(k == KC - 1))
            for j, (m, t) in enumerate(chains):
                nc.scalar.activation(qkT[m][:, TCH * t:TCH * (t + 1)],
                                     ps[j][:], AF.Identity,
                                     bias=bqk[:, m:m + 1])

        wave(W1)
        wave(W2)

        load = {"act": 0.0, "dve": 0.0}

        # ---- v chains: shared-stationary k-steps, pa on qps, pb on dum
        def v_steps(i):
            box = {}

            def step(k):
                def go():
                    if "pa" not in box:
                        box["pa"] = qps.tile([P, TCH], F32, tag="qkvps",
                                             name="vps_a")
                        box["pb"] = dum_ps.tile([P, TCH], F32, tag="dum",
                                                name="vps_b")
                    xs = xT[k][:, P * i:P * (i + 1)]
                    nc.tensor.matmul(box["pa"][:], xs, wv[k][:, 0:TCH],
                                     start=(k == 0), stop=(k == KC - 1))
                    nc.tensor.matmul(box["pb"][:, 0:C - TCH], xs,
                                     wv[k][:, TCH:C],
                                     start=(k == 0), stop=(k == KC - 1))
                    if k == KC - 1:
                        nc.vector.tensor_tensor(
                            v_sb[i][:, 0:TCH], box["pa"][:],
                            bv[:, 0:TCH], ALU.add)
                        nc.vector.tensor_tensor(
                            v_sb[i][:, TCH:C], box["pb"][:, 0:C - TCH],
                            bv[:, TCH:C], ALU.add)
                        load["dve"] += 1060
                return go
            return [step(k) for k in range(KC)]

        # v0-3 must be ready before the first AV (window 2)
        for i in range(4):
            for go in v_steps(i):
                go()

        # qk t1 fill chains (single bank each, alternating qps/dum)
        def qk_steps(m, t, use_dum):
            box = {}

            def step(k):
                def go():
                    if "ps" not in box:
                        pool, tag = ((dum_ps, "dum") if use_dum
                                     else (qps, "qkvps"))
                        box["ps"] = pool.tile([P, TCH], F32, tag=tag,
                                              name="qkfill")
                    nc.tensor.matmul(
                        box["ps"][:], wslice(k, m),
                        xT[k][:, TCH * t:TCH * (t + 1)],
                        start=(k == 0), stop=(k == KC - 1))
                    if k == KC - 1:
                        nc.scalar.activation(
                            qkT[m][:, TCH * t:TCH * (t + 1)],
                            box["ps"][:], AF.Identity,
                            bias=bqk[:, m:m + 1])
                        load["act"] += 690
                return go
            return [step(k) for k in range(KC)]

        # fill inventory: v4-7 first (deadline: av((0,1)) at window 7),
        # then the remaining qk chains in window order
        fills = []
        for i in range(4, NT):
            fills.extend(v_steps(i))
        QK_FILL = [(11, 0), (2, 1), (8, 1), (3, 1), (9, 1),
                   (4, 1), (10, 1), (5, 1), (11, 1)]
        for fidx, (m, t) in enumerate(QK_FILL):
            fills.extend(qk_steps(m, t, use_dum=(fidx % 2 == 0)))
        fi = 0

        # ======= attention windows =======
        def relu_piece(at, ps, n, diag):
            act_c = n * 0.833 + 260
            dve_c = n * 1.042 + 130
            if not diag:
                if load["act"] + act_c <= load["dve"] + dve_c:
                    load["act"] += act_c
                    nc.scalar.activation(at[:, 0:n], ps[:, 0:n],
                                         AF.Relu, scale=SCALE)
                else:
                    load["dve"] += dve_c
                    nc.vector.tensor_scalar(
                        at[:, 0:n], ps[:, 0:n], SCALE, 0.0,
                        ALU.mult, ALU.max)
                return
            if load["dve"] + dve_c <= load["act"] + act_c + 200:
                load["dve"] += dve_c
                nc.vector.scalar_tensor_tensor(
                    at[:, 0:n], ps[:, 0:n], 0.0,
                    mask[:, 0:n], ALU.max, ALU.mult)
            else:
                load["act"] += act_c
                load["dve"] += 200
                nc.scalar.activation(at[:, 0:n], ps[:, 0:n],
                                     AF.Relu, scale=SCALE)
                nc.vector.tensor_tensor(at[:, 0:P], at[:, 0:P],
                                        wedge[:], ALU.mult)

        def make_chunk(p, c):
            qt, kt = qkT[p], qkT[NPAIR + p]
            c_lo, c_hi = TCH * c, TCH * (c + 1)
            nstr = 4 * c + 4
            pieces = {}

            def emit_score(r):
                q0 = max(P * r, c_lo)
                n = c_hi - q0
                diag = P * r >= c_lo
                prs = []
                for hh in range(2):
                    h0 = HS * hh
                    ps = a_ps.tile([P, TCH], F32, tag="aps", name="aps")
                    nc.tensor.matmul(
                        ps[:, 0:n],
                        kt[h0:h0 + HS, P * r:P * (r + 1)],
                        qt[h0:h0 + HS, q0:c_hi],
                        start=True, stop=True,
                        tile_position=(h0, 0))
                    at = at_pool.tile([P, TCH], BF16, tag="attp",
                                      name="attp")
                    relu_piece(at, ps, n, diag)
                    prs.append((q0 - c_lo, n, at))
                pieces[r] = prs

            state = {"yp": None}

            def emit_av(r):
                if state["yp"] is None:
                    state["yp"] = [y_ps.tile([P, TCH], F32, tag="yps",
                                             name="yps") for _ in range(2)]
                yp = state["yp"]
                for hh in range(2):
                    h0 = HS * hh
                    off, n, at = pieces[r][hh]
                    nc.tensor.matmul(
                        yp[hh][h0:h0 + HS, off:off + n],
                        v_sb[r][:, P * p + h0:P * p + h0 + HS],
                        at[:, 0:n],
                        start=(r == 0), stop=(r == nstr - 1),
                        tile_position=(0, h0))

            def emit_wb(final=False):
                yp = state["yp"]
                c_lo_ = TCH * c
                if not final:
                    yt = yt_pool.tile([P, TCH], BF16, tag="yT", name="yT")
                    nc.scalar.activation(yt[0:HS, :], yp[0][0:HS, :],
                                         AF.Copy)
                    nc.vector.tensor_copy(yt[HS:P, :], yp[1][HS:P, :])
                    load["act"] += 687
                    load["dve"] += 660
                    (nc.sync if c == 0 else nc.scalar).dma_start(
                        y_d[P * p:P * (p + 1), c_lo_:c_lo_ + TCH],
                        yt[:, 0:TCH])
                else:
                    # 4 independent tiles -> parallel copies + DMAs
                    for q in range(4):
                        s0, s1 = P * q, P * (q + 1)
                        yt = yt_pool.tile([P, P], BF16, tag="yTf",
                                          name="yTf")
                        nc.scalar.activation(yt[0:HS, :],
                                             yp[0][0:HS, s0:s1], AF.Copy)
                        nc.vector.tensor_copy(yt[HS:P, :],
                                              yp[1][HS:P, s0:s1])
                        (nc.sync if q % 2 == 0 else nc.scalar).dma_start(
                            y_d[P * p:P * (p + 1), c_lo_ + s0:c_lo_ + s1],
                            yt[:, 0:P])

            return list(range(nstr)), emit_score, emit_av, emit_wb

        # chunk stream: all c=0 first, end on the small 4-strip (5,0)
        chunks = [(p, 0) for p in range(NPAIR - 1)]
        chunks += [(p, 1) for p in range(NPAIR)]
        chunks.append((NPAIR - 1, 0))

        prev = None  # (strips, emit_av, emit_wb)
        for (p, c) in chunks:
            strips, emit_score, emit_av, emit_wb = make_chunk(p, c)
            # score run with fills interleaved (2 per strip while any)
            for r in strips:
                emit_score(r)
                for _ in range(2):
                    if fi < len(fills):
                        fills[fi]()
                        fi += 1
            # previous chunk's AV as one contiguous accumulation run
            if prev:
                for r in prev[0]:
                    prev[1](r)
                prev[2]()
            prev = (strips, emit_av, emit_wb)
        while fi < len(fills):
            fills[fi]()
            fi += 1
        # drain last chunk
        for r in prev[0]:
            prev[1](r)
        prev[2](final=True)


def _ensure_ntff_hook():
    """Register the axon NTFF profiling hook if the image's antenv lacks
    axon_hooks (bass_utils hard-imports it on the trace=True path)."""
    import types
    try:
        from antenv import axon_hooks  # noqa: F401
        return
    except ImportError:
        pass
    import antenv
    mod = types.ModuleType("antenv.axon_hooks")
    mod._hook = None

    def set_axon_ntff_profile_hook(h):
        mod._hook = h

    def get_axon_ntff_profile_hook():
        return mod._hook

    mod.set_axon_ntff_profile_hook = set_axon_ntff_profile_hook
    mod.get_axon_ntff_profile_hook = get_axon_ntff_profile_hook
    sys.modules["antenv.axon_hooks"] = mod
    antenv.axon_hooks = mod
    try:
        from trn_agent_boot.trn_boot import _ntff_profile_via_ctypes
        hook = _ntff_profile_via_ctypes("/opt/axon/libaxon_pjrt.so")
        if hook is not None:
            mod._hook = hook
    except Exception:
        pass


_NC_CACHE = None


def _get_nc():
    global _NC_CACHE
    if _NC_CACHE is None:
        _NC_CACHE = build_nc()
    return _NC_CACHE


def kernel(x, W_attn, b_attn, _trace=False):
    import ml_dtypes
    x = np.asarray(x).astype(ml_dtypes.bfloat16)
    xt = np.ascontiguousarray(x.transpose(0, 2, 1))  # [B, C, T]
    w = np.ascontiguousarray(np.asarray(W_attn).astype(ml_dtypes.bfloat16))
    b = np.ascontiguousarray(np.asarray(b_attn, dtype=np.float32))
    assert xt.shape == (B, C, T) and w.shape == (C, 3 * C) and b.shape == (3 * C,)

    if _trace:
        _ensure_ntff_hook()
    nc = _get_nc()
    in_maps = [{"x": xt[i], "w": w, "b": b} for i in range(B)]
    res = bass_utils.run_bass_kernel_spmd(
        nc, in_maps, core_ids=list(range(B)), trace=_trace)
    y = np.stack([np.asarray(res.results[i]["y"]).astype(np.float32).T
                  for i in range(B)], axis=0)
    if _trace:
        kernel.last_result = res
    return y
